# revision 1
# baseline (speedup 1.0000x reference)
"""Trainium2 Bass kernel for a (buggy-but-well-defined) ConvTranspose2d.

Math (matches the reference exactly):
  out[b, co, i, j] = sum_{ci,kh,kw} ker[ci,co,3-kh,3-kw] * xpad[b,ci,i+kh,j+kw]
                     + bias_sum * cnt[i] * cnt[j]          for i,j in [0,66)
  out is zero elsewhere in the (B,128,126,126) output.
  xpad = x[:, :, :63, :63] zero-padded by 3 on every side -> (69,69).
  cnt  = conv(ones(63), ones(4)) = [1,2,3,4,...,4,3,2,1]  (len 66)

Strategy: data-parallel over batch (2 items / core on 8 cores).  Per core,
16 shifted 128x128 matmuls (contraction over ci on the partition dim)
accumulate each group of <=7 output rows (N = R*66 <= 462) into one PSUM
bank, plus one rank-1 K=1 matmul that adds the bias field.  Matmuls run in
the fp32r dtype (fp32 with the mantissa RNE-rounded to 11 bits — the PE's
single-pass fp32 mode, 4x the throughput of plain fp32); operands are
pre-rounded to the fp32r encoding host-side so they can be DMA'd straight
into fp32r SBUF tiles.  Weights and the padded x are shipped as one merged,
host-prepared tensor so each matmul depends on a single DMA; the mostly-zero
full output is assembled host-side.
"""

import numpy as np

import concourse.bacc as bacc
import concourse.mybir as mybir
import concourse.tile as tile
from concourse.bass_utils import run_bass_kernel_spmd

B, CIN, COUT, K, H, W = 16, 128, 128, 4, 64, 64
NCORES = 8
BPC = B // NCORES          # batch items per core
HV = H - 1                 # 63 valid input rows/cols
HP = HV + 2 * (K - 1)      # 69 padded
HO = HV + K - 1            # 66 output rows/cols (nonzero region)
HOUT = (H - 1) * 2         # 126 full output rows/cols
NWT = K * K * COUT         # 2048 weight cols
NXP = HP * HP              # 4761 padded-image cols per batch item
NXW = NWT + BPC * NXP      # merged wt+xpad tensor cols
NBF = HO * HO + COUT       # bias-field input: 66*66 field + 128 ones
F32 = mybir.dt.float32
F32R = mybir.dt.float32r

# Output row groups: (start_row, n_rows).  Grouped in two halves of 5 so at
# most 5 PSUM accumulation groups are live at once and each tap's weights are
# reused across 5 consecutive matmuls.  All N = R*66 >= 256 (full-rate f32r).
GROUPS = [(0, 7), (7, 7), (14, 7), (21, 7), (28, 5),
          (33, 7), (40, 7), (47, 7), (54, 7), (61, 5)]

_CACHE = {}


def _build_nc():
    # Bacc (not raw Bass): its finalize() legalizes sync waits — moving
    # excess matmul waits onto LDWEIGHTS and splitting multi-waits onto
    # EventSemaphore instructions — which walrus codegen requires.
    nc = bacc.Bacc(None)
    xw = nc.dram_tensor("xw", [CIN, NXW], F32R, kind="ExternalInput")
    bf = nc.dram_tensor("bf", [NBF], F32R, kind="ExternalInput")
    out = nc.dram_tensor("out", [BPC, COUT, HO, HO], F32, kind="ExternalOutput")

    with tile.TileContext(nc) as tc:
        with (
            tc.tile_pool(name="xwpool", bufs=1) as xwpool,
            tc.tile_pool(name="cpool", bufs=1) as cpool,
            tc.tile_pool(name="acc", bufs=8, space="PSUM") as psum_pool,
            tc.tile_pool(name="opool", bufs=4) as opool,
        ):
            xwt = xwpool.tile([CIN, NXW], F32R)
            # Chunked input load so the first matmuls start as soon as the
            # weights + the first half of batch 0's image have landed:
            # [weights | b0 rows 0..38 | b0 rows 39..68 | b1 image].
            c1 = NWT + 39 * HP
            nc.sync.dma_start(xwt[:, :NWT], xw[:, :NWT])
            nc.sync.dma_start(xwt[:, NWT:c1], xw[:, NWT:c1])
            nc.sync.dma_start(xwt[:, c1:NWT + NXP], xw[:, c1:NWT + NXP])
            nc.sync.dma_start(xwt[:, NWT + NXP:], xw[:, NWT + NXP:])

            bft = cpool.tile([1, NBF], F32R)
            nc.sync.dma_start(bft[:1, :], bf[None, :])
            ones = bft[0:1, HO * HO:]

            xv = xwt[:, NWT:].rearrange("p (b h w) -> p b h w",
                                        b=BPC, h=HP, w=HP)

            for b in range(BPC):
                for half in range(2):
                    groups = GROUPS[half * 5:(half + 1) * 5]
                    ptiles = {}
                    for i0, r in groups:
                        ptiles[i0] = psum_pool.tile([COUT, 7 * HO], F32,
                                                    tag="acc", name="acc")
                    for t in range(K * K):
                        kh, kw = divmod(t, K)
                        lhsT = xwt[:, t * COUT:(t + 1) * COUT]
                        for i0, r in groups:
                            rhs = xv[:, b, i0 + kh:i0 + kh + r, kw:kw + HO]
                            nc.tensor.matmul(ptiles[i0][:, :r * HO], lhsT, rhs,
                                             start=(t == 0), stop=False)
                            if t == K * K - 1:
                                # Close the group immediately after its last
                                # tap so the PSUM->SBUF copy and out-DMA of
                                # early groups overlap the remaining matmuls.
                                rb = bft[0:1, i0 * HO:(i0 + r) * HO]
                                nc.tensor.matmul(ptiles[i0][:, :r * HO], ones,
                                                 rb, start=False, stop=True)
                                otile = opool.tile([COUT, 7 * HO], F32,
                                                   tag="ot", name="ot")
                                nc.vector.tensor_copy(otile[:, :r * HO],
                                                      ptiles[i0][:, :r * HO])
                                nc.sync.dma_start(out[b, :, i0:i0 + r, :],
                                                  otile[:, :r * HO])
    nc.finalize()
    return nc


def get_nc():
    if "nc" not in _CACHE:
        _CACHE["nc"] = _build_nc()
    return _CACHE["nc"]


def _fp32r(a):
    """RNE-round fp32 -> the PE's fp32r encoding (11-bit mantissa, same 4B).

    Bit-exact with libwalrus's fp32_to_fp32r (verified on 2e5 random values).
    """
    u = np.ascontiguousarray(a, dtype=np.float32).view(np.uint32)
    r = (u + np.uint32(0x7FF) + ((u >> np.uint32(12)) & np.uint32(1))) \
        & np.uint32(0xFFFFF000)
    return r.view(np.float32)


def prep_inputs(x, kernel, bias):
    """Host-side prep: per-core input maps (numpy only, negligible cost)."""
    x = _fp32r(np.asarray(x, dtype=np.float32))
    ker = np.asarray(kernel, dtype=np.float32)
    bias = np.asarray(bias, dtype=np.float32)

    kf = ker[:, :, ::-1, ::-1]                        # [ci, co, kh, kw] flipped
    wt = _fp32r(np.ascontiguousarray(kf.transpose(0, 2, 3, 1)).reshape(
        CIN, NWT))                                    # [ci, (kh kw co)]

    cnt = np.convolve(np.ones(HV, np.float32), np.ones(K, np.float32))
    bias_sum = np.sum(bias[:COUT], dtype=np.float32)
    bfield = np.empty(NBF, np.float32)
    bfield[:HO * HO] = (bias_sum * np.outer(cnt, cnt)).astype(np.float32).ravel()
    bfield[HO * HO:] = 1.0
    bfield = _fp32r(bfield)

    in_maps = []
    for c in range(NCORES):
        xw = np.zeros((CIN, NXW), np.float32)
        xw[:, :NWT] = wt
        xp = xw[:, NWT:].reshape(CIN, BPC, HP, HP)
        # x is already fp32r-rounded; zeros are fp32r-clean.
        xp[:, :, K - 1:K - 1 + HV, K - 1:K - 1 + HV] = \
            x[c * BPC:(c + 1) * BPC, :, :HV, :HV].transpose(1, 0, 2, 3)
        in_maps.append({"xw": xw, "bf": bfield})
    return in_maps


def assemble(per_core_outs):
    out = np.zeros((B, COUT, HOUT, HOUT), np.float32)
    for c, o in enumerate(per_core_outs):
        out[c * BPC:(c + 1) * BPC, :, :HO, :HO] = o
    return out


def run(inputs, **spmd_kwargs):
    """Returns (full_output, BassKernelResults)."""
    nc = get_nc()
    in_maps = prep_inputs(**inputs)
    res = run_bass_kernel_spmd(nc, in_maps, list(range(NCORES)), **spmd_kwargs)
    return assemble([r["out"] for r in res.results]), res


def kernel(**inputs):
    out, _ = run(inputs)
    return out



# revision 2
# speedup vs baseline: 1.1794x; 1.1794x over previous
"""Trainium2 Bass kernel for a (buggy-but-well-defined) ConvTranspose2d.

Math (matches the reference exactly):
  out[b, co, i, j] = sum_{ci,kh,kw} ker[ci,co,3-kh,3-kw] * xpad[b,ci,i+kh,j+kw]
                     + bias_sum * cnt[i] * cnt[j]          for i,j in [0,66)
  out is zero elsewhere in the (B,128,126,126) output.
  xpad = x[:, :, :63, :63] zero-padded by 3 on every side.
  cnt  = conv(ones(63), ones(4)) = [1,2,3,4,...,4,3,2,1]  (len 66)

Strategy: data-parallel over batch (2 items / core on 8 cores), bf16.
Per core, per image, 10 groups of <=7 output rows; each group accumulates
its 16 shifted 128x128xN matmuls (contraction over ci) into one PSUM bank.
bf16 runs the PE at 1 col/cycle (same as fp32r) but halves DMA and gets
fast weight loads.  Rows are only padded horizontally (3 zero cols each
side); vertical padding is avoided by trimming each tap's row range, and
the first tap of each group is chosen to cover the full PSUM region so
start=True clears it.  The bias field is replicated across partitions
on-chip by K=1 fp32r matmuls during the input-DMA window (these double as
PE warm-up for the HAM clock gate, together with a few junk matmuls on a
memset tile).  Group close = one DVE tensor_add (PSUM + bias -> bf16 out
tile) followed by the output DMA.  The mostly-zero full output is
assembled host-side.
"""

import ml_dtypes
import numpy as np

import concourse.bacc as bacc
import concourse.mybir as mybir
import concourse.tile as tile
from concourse.bass_utils import run_bass_kernel_spmd

B, CIN, COUT, K, H, W = 16, 128, 128, 4, 64, 64
NCORES = 8
BPC = B // NCORES          # batch items per core
HV = H - 1                 # 63 valid input rows/cols
RS = HV + 2 * (K - 1)      # 69: row stride (cols padded by 3 each side)
HO = HV + K - 1            # 66 output rows/cols (nonzero region)
HOUT = (H - 1) * 2         # 126 full output rows/cols
NWT = K * K * COUT         # 2048 weight cols
NXI = HV * RS              # 4347 cols per image (63 rows x 69 padded cols)
NXW = NWT + BPC * NXI      # merged wt+img tensor cols
NBF = HO * HO + COUT       # bias-field input: 66*66 field + 128 ones
F32 = mybir.dt.float32
F32R = mybir.dt.float32r
BF16 = mybir.dt.bfloat16

GROUPS = [(0, 7), (7, 7), (14, 7), (21, 7), (28, 5),
          (33, 7), (40, 7), (47, 7), (54, 7), (61, 5)]

# Tap layout order in the weight tensor: kh=3 first so group 0 (whose
# full-coverage tap is kh=3) can start on the first small weight chunk.
KH_LAYOUT = [3, 0, 1, 2]
TAP_COL = {}
for _i, _kh in enumerate(KH_LAYOUT):
    for _kw in range(K):
        TAP_COL[(_kh, _kw)] = (_i * K + _kw) * COUT

# Input-image DMA row chunks (per image).
B0_CHUNKS = [(0, 7), (7, 16), (23, 16), (39, 24)]
B1_CHUNKS = [(0, 21), (21, 21), (42, 21)]

NWARM = 8                  # junk warm-up matmuls (HAM clock-gate)

_CACHE = {}


def _kh_order(i0, r):
    """Tap row order for a group: a full-row-coverage kh first (its kw=0
    matmul carries start=True and must clear the whole PSUM region)."""
    def full(kh):
        return 3 - kh - i0 <= 0 and 66 - kh - i0 >= r
    return sorted(range(K), key=lambda kh: not full(kh))


def _build_nc():
    # Bacc (not raw Bass): its finalize() legalizes sync waits — moving
    # excess matmul waits onto LDWEIGHTS and splitting multi-waits onto
    # EventSemaphore instructions — which walrus codegen requires.
    nc = bacc.Bacc(None)
    xw = nc.dram_tensor("xw", [CIN, NXW], BF16, kind="ExternalInput")
    bf = nc.dram_tensor("bf", [NBF], F32R, kind="ExternalInput")
    out = nc.dram_tensor("out", [BPC, COUT, HO, HO], BF16,
                         kind="ExternalOutput")

    with tile.TileContext(nc) as tc:
        with (
            tc.tile_pool(name="xwpool", bufs=1) as xwpool,
            tc.tile_pool(name="cpool", bufs=1) as cpool,
            tc.tile_pool(name="bspool", bufs=1) as bspool,
            tc.tile_pool(name="warm", bufs=1) as warmpool,
            tc.tile_pool(name="wps", bufs=1, space="PSUM") as warmpsum,
            tc.tile_pool(name="bps", bufs=2, space="PSUM") as biaspsum,
            tc.tile_pool(name="acc", bufs=4, space="PSUM") as psum_pool,
            tc.tile_pool(name="opool", bufs=4) as opool,
        ):
            # PE warm-up fodder: zeros, no DMA dependency.
            warmt = warmpool.tile([CIN, 462], BF16)
            nc.gpsimd.memset(warmt, 0.0)

            # Bias field + ones (tiny, lands first).
            bft = cpool.tile([1, NBF], F32R)
            nc.sync.dma_start(bft[:1, :], bf[None, :])

            # Weights (kh=3 block first), then image row chunks in the
            # order the group loop consumes them.
            xwt = xwpool.tile([CIN, NXW], BF16)
            nc.sync.dma_start(xwt[:, :4 * COUT], xw[:, :4 * COUT])
            xv = xwt[:, NWT:].rearrange("p (b r c) -> p b r c",
                                        b=BPC, r=HV, c=RS)
            xwsrc = xw[:, NWT:].rearrange("p (b r c) -> p b r c",
                                          b=BPC, r=HV, c=RS)

            def chunk(b, r0, nr):
                nc.sync.dma_start(xv[:, b, r0:r0 + nr, :],
                                  xwsrc[:, b, r0:r0 + nr, :])

            chunk(0, *B0_CHUNKS[0])
            nc.sync.dma_start(xwt[:, 4 * COUT:NWT], xw[:, 4 * COUT:NWT])
            for c in B0_CHUNKS[1:]:
                chunk(0, *c)
            for c in B1_CHUNKS:
                chunk(1, *c)

            # Junk matmuls: keep the PE busy from ~5us so the HAM clock
            # gate is at full rate when real work arrives.
            wps = warmpsum.tile([COUT, 462], F32)
            for _ in range(NWARM):
                nc.tensor.matmul(wps, warmt[:, :CIN], warmt[:, :462],
                                 start=True, stop=True)

            # Replicate the bias field across all 128 partitions:
            # ones[1,128].T @ bf[1, N] per group (fp32r, 1 col/cycle).
            bias_sb = bspool.tile([COUT, HO * HO], F32)
            ones = bft[0:1, HO * HO:]
            for i0, r in GROUPS:
                bp = biaspsum.tile([COUT, 462], F32, tag="bp", name="bp")
                nc.tensor.matmul(bp[:, :r * HO], ones,
                                 bft[0:1, i0 * HO:(i0 + r) * HO],
                                 start=True, stop=True)
                nc.vector.tensor_copy(bias_sb[:, i0 * HO:(i0 + r) * HO],
                                      bp[:, :r * HO])

            # Main conv stream: groups outer, taps inner.  Edge taps trim
            # away all-zero padded rows (the x tensor has no vertical
            # padding), writing a row-subrange of the group's PSUM bank.
            for b in range(BPC):
                for i0, r in GROUPS:
                    acc = psum_pool.tile([COUT, 462], F32,
                                         tag="acc", name="acc")
                    order = _kh_order(i0, r)
                    first = True
                    for kh in order:
                        di0 = max(0, 3 - kh - i0)
                        di1 = min(r, 66 - kh - i0)
                        x0 = i0 + kh - 3 + di0
                        for kw in range(K):
                            lhsT = xwt[:, TAP_COL[(kh, kw)]:
                                       TAP_COL[(kh, kw)] + COUT]
                            rhs = xv[:, b, x0:x0 + di1 - di0, kw:kw + HO]
                            last = kh == order[-1] and kw == K - 1
                            nc.tensor.matmul(acc[:, di0 * HO:di1 * HO],
                                             lhsT, rhs,
                                             start=first, stop=last)
                            first = False
                    otile = opool.tile([COUT, 462], BF16, tag="ot", name="ot")
                    nc.vector.tensor_add(otile[:, :r * HO], acc[:, :r * HO],
                                         bias_sb[:, i0 * HO:(i0 + r) * HO])
                    nc.sync.dma_start(out[b, :, i0:i0 + r, :],
                                      otile[:, :r * HO])
    nc.finalize()
    return nc


def get_nc():
    if "nc" not in _CACHE:
        _CACHE["nc"] = _build_nc()
    return _CACHE["nc"]


def _fp32r(a):
    """RNE-round fp32 -> the PE's fp32r encoding (11-bit mantissa)."""
    u = np.ascontiguousarray(a, dtype=np.float32).view(np.uint32)
    r = (u + np.uint32(0x7FF) + ((u >> np.uint32(12)) & np.uint32(1))) \
        & np.uint32(0xFFFFF000)
    return r.view(np.float32)


def prep_inputs(x, kernel, bias):
    """Host-side prep: per-core input maps (numpy only, negligible cost)."""
    x = np.asarray(x, dtype=np.float32)
    ker = np.asarray(kernel, dtype=np.float32)
    bias = np.asarray(bias, dtype=np.float32)

    kf = ker[:COUT, :, ::-1, ::-1]                    # [ci, co, kh, kw] flipped
    wt = np.empty((CIN, NWT), ml_dtypes.bfloat16)
    for kh in range(K):
        for kw in range(K):
            c = TAP_COL[(kh, kw)]
            wt[:, c:c + COUT] = kf[:, :, kh, kw].astype(ml_dtypes.bfloat16)

    cnt = np.convolve(np.ones(HV, np.float32), np.ones(K, np.float32))
    bias_sum = np.sum(bias[:COUT], dtype=np.float32)
    bfield = np.empty(NBF, np.float32)
    bfield[:HO * HO] = (bias_sum * np.outer(cnt, cnt)).astype(np.float32).ravel()
    bfield[HO * HO:] = 1.0
    bfield = _fp32r(bfield)

    xb = x[:, :, :HV, :HV].astype(ml_dtypes.bfloat16)
    in_maps = []
    for c in range(NCORES):
        xwm = np.zeros((CIN, NXW), ml_dtypes.bfloat16)
        xwm[:, :NWT] = wt
        xp = xwm[:, NWT:].reshape(CIN, BPC, HV, RS)
        xp[:, :, :, K - 1:K - 1 + HV] = \
            xb[c * BPC:(c + 1) * BPC].transpose(1, 0, 2, 3)
        in_maps.append({"xw": xwm, "bf": bfield})
    return in_maps


def assemble(per_core_outs):
    out = np.zeros((B, COUT, HOUT, HOUT), np.float32)
    for c, o in enumerate(per_core_outs):
        out[c * BPC:(c + 1) * BPC, :, :HO, :HO] = np.asarray(o, np.float32)
    return out


def run(inputs, **spmd_kwargs):
    """Returns (full_output, BassKernelResults)."""
    nc = get_nc()
    in_maps = prep_inputs(**inputs)
    res = run_bass_kernel_spmd(nc, in_maps, list(range(NCORES)), **spmd_kwargs)
    return assemble([r["out"] for r in res.results]), res


def kernel(**inputs):
    out, _ = run(inputs)
    return out


# revision 3
# speedup vs baseline: 1.2853x; 1.0898x over previous
"""Trainium2 Bass kernel for a (buggy-but-well-defined) ConvTranspose2d.

Math (matches the reference exactly):
  out[b, co, i, j] = sum_{ci,kh,kw} ker[ci,co,3-kh,3-kw] * xpad[b,ci,i+kh,j+kw]
                     + bias_sum * cnt[i] * cnt[j]          for i,j in [0,66)
  out is zero elsewhere in the (B,128,126,126) output.
  xpad = x[:, :, :63, :63] zero-padded by 3 on every side.
  cnt  = conv(ones(63), ones(4)) = [1,2,3,4,...,4,3,2,1]  (len 66)

Strategy: data-parallel over batch (2 items / core on 8 cores), bf16.
Per core, per image, 10 groups of <=7 output rows; each group accumulates
its 16 shifted 128x128xN matmuls (contraction over ci) into one PSUM bank.
The PE stream is pure bf16 (1 col/cycle, fast FWL weight loads, no
fp32-mode switches).  x is shipped with horizontal padding only; each
group's first matmul (a full-row-coverage tap) covers the whole PSUM
region with start=True, and every other tap is trimmed to the rows AND
columns that touch real data (per-element PSUM has_written bits make the
partial accumulation well-defined).  The bias field, replicated across
all 128 partitions, is shipped from the host and fused into the
PSUM-drain as one DVE tensor_add per group (PSUM + bias -> bf16 out
tile).  DMA issue is spread over three engines (sync: weights + output,
scalar: image chunks, gpsimd: bias field) so descriptor-issue latency
doesn't serialize the startup.  The mostly-zero full output is assembled
host-side.
"""

import ml_dtypes
import numpy as np

import concourse.bacc as bacc
import concourse.mybir as mybir
import concourse.tile as tile
from concourse.bass_utils import run_bass_kernel_spmd

B, CIN, COUT, K, H, W = 16, 128, 128, 4, 64, 64
NCORES = 8
BPC = B // NCORES          # batch items per core
HV = H - 1                 # 63 valid input rows/cols
RS = HV + 2 * (K - 1)      # 69: row stride (cols padded by 3 each side)
HO = HV + K - 1            # 66 output rows/cols (nonzero region)
HOUT = (H - 1) * 2         # 126 full output rows/cols
NWT = K * K * COUT         # 2048 weight cols
NBR = HO * HO              # 4356 replicated-bias cols
NXI = HV * RS              # 4347 cols per image (63 rows x 69 padded cols)
NXW = NWT + NBR + BPC * NXI
F32 = mybir.dt.float32
BF16 = mybir.dt.bfloat16

GROUPS = [(0, 7), (7, 7), (14, 7), (21, 7), (28, 5),
          (33, 7), (40, 7), (47, 7), (54, 7), (61, 5)]

# Tap layout order in the weight tensor: kh=3 first so group 0 (whose
# full-coverage tap is kh=3) can start on the first small weight chunk.
KH_LAYOUT = [3, 0, 1, 2]
TAP_COL = {}
for _i, _kh in enumerate(KH_LAYOUT):
    for _kw in range(K):
        TAP_COL[(_kh, _kw)] = (_i * K + _kw) * COUT

B0_CHUNKS = [(0, 7), (7, 16), (23, 16), (39, 24)]
B1_CHUNKS = [(0, 32), (32, 31)]

NWARM = 2                  # junk warm-up matmuls (HAM clock-gate)

_CACHE = {}


def _kh_order(i0, r):
    """Tap row order for a group: a full-row-coverage kh first (its kw=0
    matmul carries start=True and must clear the whole PSUM region)."""
    def full(kh):
        return 3 - kh - i0 <= 0 and 66 - kh - i0 >= r
    return sorted(range(K), key=lambda kh: not full(kh))


def _build_nc():
    # Bacc (not raw Bass): its finalize() legalizes sync waits — moving
    # excess matmul waits onto LDWEIGHTS and splitting multi-waits onto
    # EventSemaphore instructions — which walrus codegen requires.
    nc = bacc.Bacc(None)
    xw = nc.dram_tensor("xw", [CIN, NXW], BF16, kind="ExternalInput")
    out = nc.dram_tensor("out", [BPC, COUT, HO, HO], BF16,
                         kind="ExternalOutput")

    with tile.TileContext(nc) as tc:
        with (
            tc.tile_pool(name="xwpool", bufs=1) as xwpool,
            tc.tile_pool(name="warm", bufs=1) as warmpool,
            tc.tile_pool(name="wps", bufs=1, space="PSUM") as warmpsum,
            tc.tile_pool(name="acc", bufs=4, space="PSUM") as psum_pool,
            tc.tile_pool(name="opool", bufs=4) as opool,
        ):
            # PE warm-up fodder: zeros, no DMA dependency.
            warmt = warmpool.tile([CIN, 462], BF16)
            nc.gpsimd.memset(warmt, 0.0)

            xwt = xwpool.tile([CIN, NXW], BF16)
            brt = xwt[:, NWT:NWT + NBR]
            xv = xwt[:, NWT + NBR:].rearrange("p (b r c) -> p b r c",
                                              b=BPC, r=HV, c=RS)
            xwsrc = xw[:, NWT + NBR:].rearrange("p (b r c) -> p b r c",
                                                b=BPC, r=HV, c=RS)

            # Weights on sync (kh=3 block first, then the rest).
            nc.sync.dma_start(xwt[:, :4 * COUT], xw[:, :4 * COUT])
            nc.sync.dma_start(xwt[:, 4 * COUT:NWT], xw[:, 4 * COUT:NWT])
            # Replicated bias field on gpsimd (needed by the first group
            # close, ~3us after the first matmul).
            nc.gpsimd.dma_start(brt, xw[:, NWT:NWT + NBR])
            # Image row chunks on scalar, in consumption order.
            for r0, nr in B0_CHUNKS:
                nc.scalar.dma_start(xv[:, 0, r0:r0 + nr, :],
                                    xwsrc[:, 0, r0:r0 + nr, :])
            for r0, nr in B1_CHUNKS:
                nc.scalar.dma_start(xv[:, 1, r0:r0 + nr, :],
                                    xwsrc[:, 1, r0:r0 + nr, :])

            # Junk matmuls: nudge the HAM clock gate while DMA lands.
            wps = warmpsum.tile([COUT, 462], F32)
            for _ in range(NWARM):
                nc.tensor.matmul(wps, warmt[:, :CIN], warmt[:, :462],
                                 start=True, stop=True)

            # Main conv stream: groups outer, taps inner.  The first tap
            # (full row coverage, kw=0) writes the whole PSUM region with
            # start=True; all other taps are trimmed to real-data rows
            # (di0:di1) and columns (63 of 66) and accumulate into a 2D
            # row/col window of the bank.
            for b in range(BPC):
                for i0, r in GROUPS:
                    acc = psum_pool.tile([COUT, 462], F32,
                                         tag="acc", name="acc")
                    av = acc.rearrange("p (r c) -> p r c", r=7, c=HO)
                    order = _kh_order(i0, r)
                    for kh in order:
                        di0 = max(0, 3 - kh - i0)
                        di1 = min(r, 66 - kh - i0)
                        x0 = i0 + kh - 3 + di0
                        for kw in range(K):
                            lhsT = xwt[:, TAP_COL[(kh, kw)]:
                                       TAP_COL[(kh, kw)] + COUT]
                            first = kh == order[0] and kw == 0
                            last = kh == order[-1] and kw == K - 1
                            if first:
                                rhs = xv[:, b, x0:x0 + r, 0:HO]
                                dst = acc[:, :r * HO]
                            else:
                                c0 = max(0, 3 - kw)
                                rhs = xv[:, b, x0:x0 + di1 - di0, 3:3 + HV]
                                dst = av[:, di0:di1, c0:c0 + HV]
                            nc.tensor.matmul(dst, lhsT, rhs,
                                             start=first, stop=last)
                    otile = opool.tile([COUT, 462], BF16, tag="ot", name="ot")
                    nc.vector.tensor_add(otile[:, :r * HO], acc[:, :r * HO],
                                         brt[:, i0 * HO:(i0 + r) * HO])
                    nc.sync.dma_start(out[b, :, i0:i0 + r, :],
                                      otile[:, :r * HO])
    nc.finalize()
    return nc


def get_nc():
    if "nc" not in _CACHE:
        _CACHE["nc"] = _build_nc()
    return _CACHE["nc"]


def prep_inputs(x, kernel, bias):
    """Host-side prep: per-core input maps (numpy only, negligible cost)."""
    x = np.asarray(x, dtype=np.float32)
    ker = np.asarray(kernel, dtype=np.float32)
    bias = np.asarray(bias, dtype=np.float32)

    kf = ker[:COUT, :, ::-1, ::-1]                    # [ci, co, kh, kw] flipped
    wt = np.empty((CIN, NWT), ml_dtypes.bfloat16)
    for kh in range(K):
        for kw in range(K):
            c = TAP_COL[(kh, kw)]
            wt[:, c:c + COUT] = kf[:, :, kh, kw].astype(ml_dtypes.bfloat16)

    cnt = np.convolve(np.ones(HV, np.float32), np.ones(K, np.float32))
    bias_sum = np.sum(bias[:COUT], dtype=np.float32)
    bfield = (bias_sum * np.outer(cnt, cnt)).astype(ml_dtypes.bfloat16).ravel()

    xb = x[:, :, :HV, :HV].astype(ml_dtypes.bfloat16)
    in_maps = []
    for c in range(NCORES):
        xwm = np.zeros((CIN, NXW), ml_dtypes.bfloat16)
        xwm[:, :NWT] = wt
        xwm[:, NWT:NWT + NBR] = bfield[None, :]
        xp = xwm[:, NWT + NBR:].reshape(CIN, BPC, HV, RS)
        xp[:, :, :, K - 1:K - 1 + HV] = \
            xb[c * BPC:(c + 1) * BPC].transpose(1, 0, 2, 3)
        in_maps.append({"xw": xwm})
    return in_maps


def assemble(per_core_outs):
    out = np.zeros((B, COUT, HOUT, HOUT), np.float32)
    for c, o in enumerate(per_core_outs):
        out[c * BPC:(c + 1) * BPC, :, :HO, :HO] = np.asarray(o, np.float32)
    return out


def run(inputs, **spmd_kwargs):
    """Returns (full_output, BassKernelResults)."""
    nc = get_nc()
    in_maps = prep_inputs(**inputs)
    res = run_bass_kernel_spmd(nc, in_maps, list(range(NCORES)), **spmd_kwargs)
    return assemble([r["out"] for r in res.results]), res


def kernel(**inputs):
    out, _ = run(inputs)
    return out
